# revision 41
# baseline (speedup 1.0000x reference)
"""Trainium2 Bass kernel for the DiffRenderer problem.

Math refactor (validated against the jax reference):
  The renderer's per-point MLP input collapses to
      a[b, pix, d, h] = depth[b, d] * g[b, pix, h] + e[b, h]
  with
      g[b] = Q @ V_b,  V_b = M_b^T @ W1[:3],  M_b = s_obj * R_obj
      e[b] = c_b @ W1[:3] + z_shape[b] @ W1[3:] + b1
      depth[b, d] = zs[d] * s_inv[b] + bb_depth[b]
  Layer 1 + relu:  h' = max(depth*g, -e)   (== relu(a) - e)
  Layer 2:         s  = W2 . h' + (W2 . e + b2)   (bias folded into tanh)
  sdf = tanh(s); then the zero-crossing depth extraction.

Sharding: 8 cores = 4 batches x 2 pixel-halves (2048 pixels/core, 64 depths).

Per-core device pipeline (all phases overlap via the Tile scheduler):
  PE:  g = Q @ V (float32r); 64x5 layer-2 matmuls with a sliding one-hot
       W2 stationary [128,64] so psum row d accumulates s[d, :] directly
       (float32r = 1 cycle/row); 16 transposes [64,128] -> [128,64]
  DVE: h' = max(depth_d * g, -e) for pixels 0..1407 (tensor_scalar
       mult+max, 2x_2p mode); pos/zc/PZ + all free-axis reduces + finals
  ACT: h' = relu(depth_d * g + e) for pixels 1408..2047 (activation with
       per-partition scale/bias); psum->sbuf copies; tanh (+bias fold)
  GpS: input-DMA issue; P1/P2 products (software Q7, ~2.3ns/elem)
HW constraints baked in (measured): GpSimd cannot touch PSUM and its fp
  tensor_tensor supports only add/sub/mult; a single shared h' tile
  with two writers stalls all queues (tile-granular WAW semaphores), so
  hp_d/hp_a stay separate; input DMA transfers land ~0.7-1.5us after
  issue, hence the 3-way qv split.
Precision: float32r (FP22 in the PE) for g and layer-2, fp32 elsewhere;
  bf16 was measured to flip sdf signs in the randn-weight regime and is
  deliberately NOT used.
"""

import os
import sys

import numpy as np

for _p in ("/opt/trn_rl_repo", "/root/.axon_site/_ro/trn_rl_repo"):
    if os.path.isdir(_p) and _p not in sys.path:
        sys.path.append(_p)

from contextlib import ExitStack

from concourse import bacc, bass, masks, mybir, tile
from concourse.bass_utils import run_bass_kernel_spmd

F32 = mybir.dt.float32
F32R = mybir.dt.float32r
ALU = mybir.AluOpType
ACTF = mybir.ActivationFunctionType

IMG = 64
D = 64
HID = 128
BS = 4
NCORES = 8
PIX = IMG * IMG          # 4096 pixels per batch
PPC = PIX // 2           # 2048 pixels per core
NT = PPC // 128          # 16 pixel tiles per core
SPLIT = 1408             # pixels 0..1407 on DVE, 1408..2047 on ACT
                         # (HW-proven 3-way DVE/ACT/PE balance; tile 11 is
                         # the first ACT-path tile)
K63 = 63                 # depth pairs per tile
NWARM = 0                # PE warmup was measured net-negative; disabled

_PROGRAM = None


def build_program():
    nc = bacc.Bacc(None, target_bir_lowering=False)
    # qs (3 x 2048 pixel rays) and vb (3 x 128) packed into one DMA
    qv = nc.declare_dram_parameter("qv", [3, PPC + HID], F32R, isOutput=False)
    w2w = nc.declare_dram_parameter("w2w", [HID, 127], F32R, isOutput=False)
    # cst cols: 0:64 depth, 64 nege, 65 epos, 66 bias0(s0), 67 bias1(b2), 68 lam
    cst = nc.declare_dram_parameter("cst", [128, 69], F32, isOutput=False)
    zm1 = nc.declare_dram_parameter("zm1", [128, NT * K63], F32, isOutput=False)
    dp_o = nc.declare_dram_parameter("dp", [128, NT], F32, isOutput=True)
    occ_o = nc.declare_dram_parameter("occ", [128, NT], F32, isOutput=True)

    with tile.TileContext(nc) as tc, ExitStack() as ctx:
        const = ctx.enter_context(tc.tile_pool(name="const", bufs=1))
        gpool = ctx.enter_context(tc.tile_pool(name="g", bufs=1))
        hpool = ctx.enter_context(tc.tile_pool(name="hp", bufs=6))
        spool = ctx.enter_context(tc.tile_pool(name="s", bufs=1))
        post = ctx.enter_context(tc.tile_pool(name="post", bufs=1))
        pss = ctx.enter_context(tc.tile_pool(name="pss", bufs=1, space="PSUM"))
        pst = ctx.enter_context(tc.tile_pool(name="pst", bufs=3, space="PSUM"))

        # qv layout: [vb | qs]; three DMAs so g matmul k waits only on the
        # transfer actually covering its qs chunk (DMA transfer completes
        # ~1.5us after issue for the full tensor; the small first piece
        # unblocks g0 ~1us earlier)
        t_qv = const.tile([3, PPC + HID], F32R, name="t_qv", tag="t_qv")
        nc.sync.dma_start(t_qv[:, 0:HID + 512], qv[:, 0:HID + 512])
        nc.sync.dma_start(t_qv[:, HID + 512:HID + 1536], qv[:, HID + 512:HID + 1536])
        nc.sync.dma_start(t_qv[:, HID + 1536:], qv[:, HID + 1536:])
        t_cst = const.tile([128, 69], F32, name="t_cst", tag="t_cst")
        nc.gpsimd.dma_start(t_cst[:], cst[:])
        w2win = const.tile([HID, 127], F32R, name="t_w2w", tag="t_w2w")
        nc.gpsimd.dma_start(w2win[:], w2w[:])
        t_zm1 = const.tile([128, NT * K63], F32, name="t_zm1", tag="t_zm1")
        nc.gpsimd.dma_start(t_zm1[:], zm1[:])

        # identity (epilogue transposes only) after the DMA issues
        ident = const.tile([64, 64], F32)
        masks.make_identity(nc, ident[:])
        # force the GpSimd TT ucode library load now (idle prologue) instead
        # of mid-epilogue
        gps_warm = const.tile([128, 1], F32, name="gps_warm", tag="gps_warm")
        nc.gpsimd.tensor_tensor(
            gps_warm[:], t_cst[:, 0:1], t_cst[:, 0:1], op=ALU.mult)

        t_vb = t_qv[:, 0:HID]
        t_qs = t_qv[:, HID:HID + PPC]
        t_depth = t_cst[:, 0:D]
        t_nege = t_cst[:, D:D + 1]
        t_epos = t_cst[:, D + 1:D + 2]
        t_bias0 = t_cst[:, D + 2:D + 3]
        t_bias1 = t_cst[:, D + 3:D + 4]
        t_lam = t_cst[:, D + 4:D + 5]

        # ---- g = Q @ V  (float32r matmul; one-time) ----
        # psum slots shared (by tag) with the layer-2 chunk tiles: the g
        # phase finishes before the d-loop's accumulators are first used.
        # copies alternate DVE/ACT (GpSimd cannot access PSUM).
        g_sb = gpool.tile([HID, PPC], F32)
        g_pgs = []
        for k in range(PPC // 512):
            pg = pss.tile([HID, 512], F32, name=f"pg{k}", tag=f"pss{k}")
            nc.tensor.matmul(
                pg[:], t_vb, t_qs[:, k * 512:(k + 1) * 512],
                start=True, stop=True,
            )
            g_pgs.append(pg)
            if k % 2 == 1:
                nc.scalar.copy(g_sb[:, k * 512:(k + 1) * 512], pg[:])

        # ---- main d-loop: h' then layer-2 (accumulating masked matmuls) ----
        # Separate hp_d/hp_a tiles per producer (a single shared tile was
        # measured 20% SLOWER on HW: tile-granular WAW/WAR semaphores couple
        # the DVE and ACT queues and stall the PE into mid p-state).
        # Chunk layout over the 2048 pixels (split DVE 1408 / ACT 640):
        CHUNKS = [(0, 512), (512, 512), (1024, 384), (1408, 384), (1792, 256)]
        NCH = len(CHUNKS)
        ps_chunks = [
            pss.tile([D, n], F32, name=f"ps_chunk{k}", tag=f"pss{k}")
            for k, (off, n) in enumerate(CHUNKS)
        ]
        for d in range(D):
            hp_d = hpool.tile([HID, SPLIT], F32R, tag="hpd")
            hp_a = hpool.tile([HID, PPC - SPLIT], F32R, tag="hpa")
            if d == 0:
                # interleave the DVE g-copies with the split d=0
                # tensor_scalars so the first layer-2 matmul starts as soon
                # as g chunk 0 is copied
                nc.vector.tensor_copy(g_sb[:, 0:512], g_pgs[0][:])
                nc.vector.tensor_scalar(
                    hp_d[:, 0:512], g_sb[:, 0:512],
                    t_depth[:, d:d + 1], t_nege[:, 0:1],
                    op0=ALU.mult, op1=ALU.max,
                )
                nc.vector.tensor_copy(g_sb[:, 1024:1536], g_pgs[2][:])
                for o0, o1 in ((512, 1024), (1024, SPLIT)):
                    nc.vector.tensor_scalar(
                        hp_d[:, o0:o1], g_sb[:, o0:o1],
                        t_depth[:, d:d + 1], t_nege[:, 0:1],
                        op0=ALU.mult, op1=ALU.max,
                    )
            else:
                nc.vector.tensor_scalar(
                    hp_d[:], g_sb[:, 0:SPLIT],
                    t_depth[:, d:d + 1], t_nege[:, 0:1],
                    op0=ALU.mult, op1=ALU.max,
                )
            nc.scalar.activation(
                hp_a[:], g_sb[:, SPLIT:PPC], ACTF.Relu,
                bias=t_epos[:, 0:1], scale=t_depth[:, d:d + 1],
            )
            w2slice = w2win[:, 63 - d:127 - d]
            for k, (off, n) in enumerate(CHUNKS):
                if off >= SPLIT:
                    srck = hp_a[:, off - SPLIT:off - SPLIT + n]
                else:
                    srck = hp_d[:, off:off + n]
                nc.tensor.matmul(
                    ps_chunks[k][:], w2slice, srck,
                    start=(d == 0), stop=(d == D - 1),
                    skip_group_check=True,
                )

        # ---- tail: per-chunk copy, transpose, tanh; then postprocess ----
        s_sb = spool.tile([D, PPC], F32)
        sdf = post.tile([128, NT * D], F32)
        pos = post.tile([128, NT * D], F32)
        zc = post.tile([128, NT * K63], F32)
        ptmp = post.tile([128, NT * K63], F32)
        p1 = post.tile([128, NT * K63], F32)
        p2 = post.tile([128, NT * K63], F32)
        pz = post.tile([128, NT * K63], F32)
        d1r = post.tile([128, NT], F32)
        s1 = post.tile([128, NT], F32)
        s2 = post.tile([128, NT], F32)
        sdf3 = sdf[:].rearrange("p (t d) -> p t d", d=D)
        pos3 = pos[:].rearrange("p (t d) -> p t d", d=D)
        zc3 = zc[:].rearrange("p (t k) -> p t k", k=K63)
        ptmp3 = ptmp[:].rearrange("p (t k) -> p t k", k=K63)
        p1_3 = p1[:].rearrange("p (t k) -> p t k", k=K63)
        p2_3 = p2[:].rearrange("p (t k) -> p t k", k=K63)
        pz_3 = pz[:].rearrange("p (t k) -> p t k", k=K63)
        zm1_3 = t_zm1[:].rearrange("p (t k) -> p t k", k=K63)

        # per-chunk: psum->sbuf copy (alternating ACT/DVE; GpSimd/DMA cannot
        # access psum), PE transposes, then tanh. Chunk boundaries align
        # with the DVE/ACT pixel split (tiles 0..10 bias s0, 11..15 bias b2).
        for k, (off, n) in enumerate(CHUNKS):
            t0, t1 = off // 128, (off + n) // 128
            if k % 2 == 0:
                nc.scalar.copy(s_sb[:, off:off + n], ps_chunks[k][:])
            else:
                nc.vector.tensor_copy(s_sb[:, off:off + n], ps_chunks[k][:])
            pt = pst.tile([128, 4 * D], F32, name=f"pt{k}", tag="pt", bufs=3)
            for j, i in enumerate(range(t0, t1)):
                nc.tensor.transpose(
                    pt[:, j * D:(j + 1) * D], s_sb[:, i * 128:(i + 1) * 128],
                    ident[:])
            b_ap = t_bias0 if k < 3 else t_bias1
            nc.scalar.activation(
                sdf[:, t0 * D:t1 * D], pt[:, 0:(t1 - t0) * D], ACTF.Tanh,
                bias=b_ap[:, 0:1], scale=1.0,
            )

        # postprocess in 2 groups of 8 tiles. Engine budget (HW-measured):
        # DVE TT/reduce ~0.68us per 8-tile pass, GpSimd TT ~1.16us (software
        # Q7), and GpSimd cannot do compares or free-axis reduces. So: DVE
        # does pos/zc/P2 + all reduces; GpSimd does the P1/PZ products.
        # pos/zc for BOTH groups are emitted before any reduce so the DVE
        # queue never blocks group-B mask work behind group-A reduces.
        GA, GB = slice(0, 8), slice(8, 16)
        # postprocess in 2 groups of 8 tiles. DVE: pos/zc/P2 + all free-axis
        # reduces (GpSimd can't do compares or free-axis reduces); GpSimd:
        # the P1/PZ products. pos/zc for both groups precede the reduces so
        # the DVE queue never blocks group-B mask work behind group-A
        # reduces. (This exact arrangement measured 90108ns; later
        # reshuffles measured worse.)
        for t0, t1 in ((0, 8), (8, 16)):
            ts_ = slice(t0, t1)
            nc.vector.tensor_scalar(
                pos[:, t0 * D:t1 * D],
                sdf[:, t0 * D:t1 * D], 0.0, None, op0=ALU.is_gt)
            nc.vector.tensor_tensor(
                zc3[:, ts_, :], pos3[:, ts_, 0:K63], pos3[:, ts_, 1:D],
                op=ALU.is_gt)
            nc.gpsimd.tensor_tensor(
                p1_3[:, ts_, :], zc3[:, ts_, :], sdf3[:, ts_, 0:K63],
                op=ALU.mult)
            nc.vector.tensor_tensor(
                p2_3[:, ts_, :], zc3[:, ts_, :], sdf3[:, ts_, 1:D],
                op=ALU.mult)
            nc.gpsimd.tensor_tensor(
                pz_3[:, ts_, :], zc3[:, ts_, :], zm1_3[:, ts_, :],
                op=ALU.mult)
        for ts_ in (GA, GB):
            nc.vector.tensor_reduce(
                s1[:, ts_], p1_3[:, ts_, :], axis=mybir.AxisListType.X,
                op=ALU.add)
            nc.vector.tensor_reduce(
                s2[:, ts_], p2_3[:, ts_, :], axis=mybir.AxisListType.X,
                op=ALU.add)
            nc.vector.tensor_reduce(
                d1r[:, ts_], pz_3[:, ts_, :], axis=mybir.AxisListType.X,
                op=ALU.min)

        # ---- global finals on [128, NT] ----
        occ_sb = post.tile([128, NT], F32)
        nc.vector.tensor_scalar(occ_sb[:], d1r[:], -50.0, None, op0=ALU.is_le)
        nc.sync.dma_start(occ_o[:], occ_sb[:])
        den = post.tile([128, NT], F32)
        nc.vector.scalar_tensor_tensor(
            den[:], s2[:], 1e-6, s1[:], op0=ALU.subtract, op1=ALU.subtract
        )
        rec = post.tile([128, NT], F32)
        nc.vector.reciprocal(rec[:], den[:])
        interp = post.tile([128, NT], F32)
        nc.vector.scalar_tensor_tensor(
            interp[:], rec[:], t_lam[:, 0:1], s1[:], op0=ALU.mult, op1=ALU.mult)
        res = post.tile([128, NT], F32)
        nc.vector.scalar_tensor_tensor(
            res[:], d1r[:], 100.0, interp[:], op0=ALU.add, op1=ALU.subtract)
        dp_sb = post.tile([128, NT], F32)
        nc.vector.tensor_tensor(dp_sb[:], occ_sb[:], res[:], op=ALU.mult)
        nc.sync.dma_start(dp_o[:], dp_sb[:])

    nc.finalize()
    return nc


def host_prep(z_shape, z_extr, W1, b1, W2, b2):
    """Per-core input maps. All small math mirrors the reference in
    float64 (deviations ~1e-7, far inside the sdf sign margins)."""
    f32 = np.float32
    z_shape = np.asarray(z_shape, f32)
    z_extr = np.asarray(z_extr, f32)
    W1 = np.asarray(W1, f32)
    b1 = np.asarray(b1, f32)
    W2 = np.asarray(W2, f32)
    b2 = np.asarray(b2, f32)

    f = 70.0 * (IMG / 64.0)
    cc = IMG / 2.0 - 0.5
    Km = np.array([[f, 0, cc], [0, f, cc], [0, 0, 1]], np.float64)
    K_inv = np.linalg.inv(Km)
    t = np.array([0.0, 0.0, 2.5])

    # mirror the reference's f32 double-reciprocal
    s_obj32 = (1.0 / z_extr[:, 0]).astype(f32)
    s_inv32 = (1.0 / s_obj32).astype(f32)
    s_obj = s_obj32.astype(np.float64)
    s_inv = s_inv32.astype(np.float64)
    t_obj = z_extr[:, 1:4].astype(np.float64)
    alpha = z_extr[:, 4].astype(np.float64)

    a = np.pi * alpha
    ca, sa = np.cos(a), np.sin(a)
    R_obj = np.zeros((BS, 3, 3))
    R_obj[:, 0, 0] = ca
    R_obj[:, 0, 1] = -sa
    R_obj[:, 1, 0] = sa
    R_obj[:, 1, 1] = ca
    R_obj[:, 2, 2] = 1.0

    corners = np.array(
        [[1, 1, 1], [1, 1, -1], [1, -1, 1], [1, -1, -1],
         [-1, 1, 1], [-1, 1, -1], [-1, -1, 1], [-1, -1, -1], [0, 0, 0]],
        np.float64,
    )
    R_obj_inv = np.linalg.inv(R_obj)
    # z-component of K @ (R_t^-1 (R_obj_inv (s_inv * corner) + t_obj) + t)
    zc = np.einsum("bij,aj->bai", R_obj_inv, corners)[:, :, 2] * s_inv[:, None]
    bb_depth = zc.mean(axis=1) + t_obj[:, 2] + 2.5      # (BS,)

    zs = np.linspace(-1.0, 1.0, D)
    depth_bd = (zs[None, :] * s_inv[:, None] + bb_depth[:, None]).astype(f32)

    M = s_obj[:, None, None] * R_obj
    c_b = np.einsum("bij,bj->bi", M, -(t[None, :] + t_obj))
    V = np.einsum("bij,ih->bjh", M, W1[:3].astype(np.float64))   # (BS,3,H)
    e = (
        np.einsum("bi,ih->bh", c_b, W1[:3].astype(np.float64))
        + z_shape.astype(np.float64) @ W1[3:].astype(np.float64)
        + b1.astype(np.float64)
    )
    e32 = e.astype(f32)
    s0 = (e32.astype(np.float64) @ W2.astype(np.float64) + b2.astype(np.float64))
    s0 = s0.astype(f32)                                  # (BS,1)

    xs = np.linspace(0.0, IMG - 1.0, IMG)
    Xg, Yg = np.meshgrid(xs, xs)
    p3 = np.stack([Xg.reshape(-1), Yg.reshape(-1), np.ones(PIX)], -1)
    q = p3 @ K_inv.T                                     # (PIX, 3)

    w2win_host = np.zeros((HID, 127), f32)
    w2win_host[:, 63] = W2[:, 0]
    in_maps = []
    for c in range(NCORES):
        b, half = c // 2, c % 2
        qs_c = q[half * PPC:(half + 1) * PPC].T.astype(f32)        # (3, PPC)
        vb_c = V[b].astype(f32)                                     # (3, H)
        qv_c = np.concatenate([vb_c, qs_c], axis=1)                 # (3, H+PPC)
        dep = np.broadcast_to(depth_bd[b], (128, D)).astype(f32)
        cst_c = np.zeros((128, 69), f32)
        cst_c[:, 0:D] = dep
        cst_c[:, D] = -e32[b]
        cst_c[:, D + 1] = e32[b]
        cst_c[:, D + 2] = s0[b, 0]
        cst_c[:, D + 3] = b2[0]
        cst_c[:, D + 4] = np.float32(depth_bd[b][1] - depth_bd[b][0])
        zrow1 = np.tile(depth_bd[b][0:K63] - 100.0, NT).astype(f32)
        in_maps.append({
            "qv": np.ascontiguousarray(qv_c),
            "w2w": w2win_host,
            "cst": cst_c,
            "zm1": np.broadcast_to(zrow1, (128, NT * K63)).copy(),
        })
    return in_maps


def _assemble(results):
    f32 = np.float32
    dp_full = np.zeros((BS, PIX), f32)
    occ_full = np.zeros((BS, PIX), f32)
    for c in range(NCORES):
        b, half = c // 2, c % 2
        sl = slice(half * PPC, (half + 1) * PPC)
        dp_full[b, sl] = np.asarray(results[c]["dp"]).T.ravel()
        occ_full[b, sl] = np.asarray(results[c]["occ"]).T.ravel()
    return (
        dp_full.reshape(BS, IMG, IMG, 1),
        occ_full.reshape(BS, IMG, IMG, 1),
    )


def get_program():
    global _PROGRAM
    if _PROGRAM is None:
        _PROGRAM = build_program()
    return _PROGRAM


def kernel(z_shape, z_extr, W1, b1, W2, b2, **run_kwargs):
    nc = get_program()
    in_maps = host_prep(z_shape, z_extr, W1, b1, W2, b2)
    res = run_bass_kernel_spmd(nc, in_maps, core_ids=list(range(NCORES)), **run_kwargs)
    out = _assemble(res.results)
    if run_kwargs:
        return out, res
    return out
